# revision 1
# baseline (speedup 1.0000x reference)
"""Trainium2 Bass kernel for a dense transformer decoder layer.

Layer: RMSNorm -> QKV(+RoPE) -> causal GQA attention -> out-proj -> residual
       -> RMSNorm -> SwiGLU FFN -> residual
Shapes: B=2, S=2048, E=2048, NH=16, NKV=4, HD=128, FF=5632, fp32 I/O.

Sharding: DP over batch (2 replicas) x TP within replica (4 cores):
  - each core owns NH/TP q-heads + 1 kv-head, QKV column-parallel
  - out-proj row-parallel -> per-token-block AllReduce over the 4-core group
  - FFN hidden dim split 4 ways, w3 row-parallel; the FFN residual (h2/TP) is
    folded into the w3 partials so a ReduceScatter directly produces this
    core's embed-row shard of the final output (SPMD-uniform addressing).
  - all on-chip activations are feature-major ("T layout": [feature, token]);
    host pre-transposes x and weights so no on-chip transposes are needed.

RMSNorm weight vectors are folded into the matmul weights on host; the
per-token 1/rms scale is computed via x^2 ones-matmuls and applied to xT
tiles on the fly.  Softmax runs without max-subtraction (logits here are
O(1); exp cannot overflow), causal masking via block skipping plus
affine_select on diagonal blocks, ones-matmul for denominators.

Matmul operand dtype is configurable (fp32 / bf16 / f32r); accumulation is
always fp32 in PSUM, and both residual paths stay fp32 end-to-end.
"""

import math
import os
import sys

import numpy as np

for _p in ("/opt/trn_rl_repo",):
    if _p not in sys.path and os.path.isdir(_p):
        sys.path.insert(0, _p)

import concourse.bass as bass
import concourse.tile as tile
from concourse import bacc, mybir
from concourse.bass_utils import run_bass_kernel_spmd

# ---------------------------------------------------------------- constants
B, S, E = 2, 2048, 2048
NH, NKV, HD, FF = 16, 4, 128, 5632
EPS = 1e-5
SM_SCALE = 1.0 / math.sqrt(float(E))  # reference scales by sqrt(embed_dim)

N_CORES = 8
TP = 4                      # tensor-parallel degree (cores per replica)
DP = N_CORES // TP          # data-parallel over batch
HL = NH // TP               # local q heads
QD = HL * HD                # local q dims
FFS = FF // TP              # local FFN hidden dims
FM = FFS // 128             # ff m-tiles
KE = E // 128               # embed k-tiles (16)
NT = 512                    # token block (matmul moving free dim)
NB = S // NT                # token blocks (4)
EM = E // 128               # embed out tiles (16)
ESH = E // TP               # output shard rows per core

FP = mybir.dt.float32
AF = mybir.ActivationFunctionType

# configuration (overridable via env for experiments)
CFG = {
    "mmdt": os.environ.get("BASS_KERNEL_MMDT", "bf16"),
    "coll_fc": os.environ.get("BASS_KERNEL_COLL_FC", "bf16"),
    "collectives": os.environ.get("BASS_KERNEL_COLL", "1") == "1",
}

_DT = {"f32": mybir.dt.float32, "bf16": mybir.dt.bfloat16,
       "f32r": mybir.dt.float32r}

_prog_cache = {}


# ------------------------------------------------------------- device program
def _build_program(limit=4, mmdt=None, coll_fc=None, collectives=None):
    mmdt = CFG["mmdt"] if mmdt is None else mmdt
    coll_fc = CFG["coll_fc"] if coll_fc is None else coll_fc
    collectives = CFG["collectives"] if collectives is None else collectives
    MD = _DT[mmdt]                       # matmul operand dtype
    WD = MD                              # weight dtype in DRAM
    CF = _DT[coll_fc]                    # fc AllReduce payload dtype

    nc = bacc.Bacc("TRN2", target_bir_lowering=False, debug=False,
                   num_devices=N_CORES)

    xT_d = nc.dram_tensor("xT", [E, S], FP, kind="ExternalInput").ap()
    xTb_d = nc.dram_tensor("xTb", [E, S], MD, kind="ExternalInput").ap()
    wqkv_d = nc.dram_tensor("wqkv", [E, QD + 2 * HD], WD,
                            kind="ExternalInput").ap()
    wfc_d = nc.dram_tensor("wfc", [QD, E], WD, kind="ExternalInput").ap()
    w1_d = nc.dram_tensor("w1", [FM, E, 128], WD, kind="ExternalInput").ap()
    w2_d = nc.dram_tensor("w2", [FM, E, 128], WD, kind="ExternalInput").ap()
    w3_d = nc.dram_tensor("w3", [EM, FFS, 128], WD, kind="ExternalInput").ap()
    cosF_d = nc.dram_tensor("cosF", [HD, S], FP, kind="ExternalInput").ap()
    sinS_d = nc.dram_tensor("sinS", [HD, S], FP, kind="ExternalInput").ap()

    out_d = nc.dram_tensor("outT", [ESH, S], FP, kind="ExternalOutput").ap()

    groups = [[r * TP + t for t in range(TP)] for r in range(DP)]

    with tile.TileContext(nc) as tc:
        with (
            tc.tile_pool(name="const", bufs=1) as const,
            tc.tile_pool(name="dram", bufs=1, space="DRAM") as dram,
        ):
            # ---- internal DRAM buffers (pool tiles => dep tracking works)
            fc_par = [dram.tile([E, NT], CF, tag=f"fcp{j}", name=f"fcp{j}")
                      for j in range(NB)]
            fc_ar = [dram.tile([E, NT], CF, tag=f"fca{j}", name=f"fca{j}")
                     for j in range(NB)]
            ff_par = [dram.tile([E, NT], FP, tag=f"ffp{j}", name=f"ffp{j}")
                      for j in range(NB)]
            rs_out = [dram.tile([ESH, NT], FP, tag=f"rso{j}", name=f"rso{j}")
                      for j in range(NB)]

            # ---- constants
            ones_md = const.tile([128, 1], MD)
            nc.vector.memset(ones_md, 1.0)
            eps_t = const.tile([1, 1], FP)
            nc.vector.memset(eps_t, EPS)
            # persistent activation stores for attention
            with (
                tc.tile_pool(name="qkstore", bufs=1) as qkstore,
                tc.tile_pool(name="vstore", bufs=1) as vstore,
            ):
                q_sb = [qkstore.tile([128, S], MD, tag=f"q{m}", name=f"q{m}")
                        for m in range(HL)]
                k_sb = qkstore.tile([128, S], MD, tag="kk")
                v_sb = [vstore.tile([128, HD], MD, tag=f"v{t}", name=f"v{t}")
                        for t in range(S // 128)]

                # ---------------- phase 0: x row-norm stats ----------------
                sbcp_cm = tc.tile_pool(name="sbcp", bufs=1)
                sbcp = sbcp_cm.__enter__()
                sbc = sbcp.tile([128, S], FP)        # 1/rms(x) broadcast
                s_row = sbcp.tile([1, S], FP)
                s_dram = dram.tile([1, S], FP, tag="s_dram", name="s_dram")
                s_col = [vstore.tile([128, 1], FP, tag=f"sc{t}",
                                     name=f"sc{t}")
                         for t in range(S // 128)]
                with (
                    tc.tile_pool(name="p0", bufs=4) as p0,
                    tc.tile_pool(name="ps0", bufs=2, space="PSUM") as ps0,
                ):
                    for j in range(NB):
                        jc = bass.ts(j, NT)
                        ps_s = ps0.tile([1, NT], FP, tag="ps_s")
                        for k in range(KE):
                            xt = p0.tile([128, NT], FP, tag="xt")
                            nc.sync.dma_start(out=xt, in_=xT_d[bass.ts(k, 128), jc])
                            sq = p0.tile([128, NT], MD, tag="sq")
                            if k % 2 == 0:
                                nc.vector.tensor_mul(sq, xt, xt)
                            else:
                                nc.scalar.activation(sq, xt, AF.Square)
                            nc.tensor.matmul(ps_s, ones_md, sq,
                                             start=(k == 0), stop=(k == KE - 1))
                        srt = p0.tile([1, NT], FP, tag="srt")
                        nc.scalar.activation(srt, ps_s, AF.Sqrt,
                                             bias=eps_t, scale=1.0 / E)
                        nc.vector.reciprocal(s_row[:, jc], srt)
                    nc.gpsimd.partition_broadcast(sbc, s_row)
                nc.sync.dma_start(out=s_dram[:], in_=s_row)
                for tt in range(S // 128):
                    nc.sync.dma_start(
                        out=s_col[tt],
                        in_=s_dram[0, bass.ts(tt, 128)].rearrange(
                            "(p one) -> p one", one=1))



                # ---------------- phase 1: qkv + rope ----------------
                with (
                    tc.tile_pool(name="wq", bufs=1) as wqp,
                    tc.tile_pool(name="xh", bufs=KE) as xhp,
                    tc.tile_pool(name="rp", bufs=3) as rp,
                    tc.tile_pool(name="ps1", bufs=2, space="PSUM") as ps1,
                    tc.tile_pool(name="ps1v", bufs=2, space="PSUM") as ps1v,
                ):
                    wq_sb = [wqp.tile([128, QD + 2 * HD], MD, tag=f"wq{k}",
                                      name=f"wq{k}")
                             for k in range(KE)]
                    for k in range(KE):
                        nc.sync.dma_start(out=wq_sb[k],
                                          in_=wqkv_d[bass.ts(k, 128), :])
                    for j in range(NB if limit >= 1 else 0):
                        jc = bass.ts(j, NT)
                        cosj = rp.tile([128, NT], FP, tag="cosj")
                        nc.sync.dma_start(out=cosj, in_=cosF_d[:, jc])
                        sinj = rp.tile([128, NT], FP, tag="sinj")
                        nc.sync.dma_start(out=sinj, in_=sinS_d[:, jc])
                        # fold the rmsnorm token scale into the rope tables
                        nc.vector.tensor_mul(cosj, cosj, sbc[:, jc])
                        nc.vector.tensor_mul(sinj, sinj, sbc[:, jc])
                        xh = []
                        for k in range(KE):
                            xhk = xhp.tile([128, NT], MD, tag="xh")
                            nc.sync.dma_start(
                                out=xhk, in_=xTb_d[bass.ts(k, 128), jc])
                            xh.append(xhk)
                        # v tiles (token-major) for 4 token-tiles of block j
                        for tt in range(4):
                            psv = ps1v.tile([128, HD], FP, tag="psv")
                            for k in range(KE):
                                nc.tensor.matmul(
                                    psv,
                                    xh[k][:, bass.ts(tt, 128)],
                                    wq_sb[k][:, QD + HD:QD + 2 * HD],
                                    start=(k == 0), stop=(k == KE - 1))
                            nc.vector.tensor_scalar_mul(
                                v_sb[4 * j + tt], psv, s_col[4 * j + tt])
                        # q (HL tiles) + k (1 tile), feature-major + rope
                        for m in range(HL + 1):
                            pq = ps1.tile([128, NT], FP, tag="pq")
                            for k in range(KE):
                                nc.tensor.matmul(
                                    pq,
                                    wq_sb[k][:, bass.ts(m, 128)],
                                    xh[k],
                                    start=(k == 0), stop=(k == KE - 1))
                            dst = (q_sb[m] if m < HL else k_sb)[:, jc]
                            qraw = rp.tile([128, NT], FP, tag="qraw")
                            nc.scalar.activation(qraw, pq, AF.Copy)
                            swp = rp.tile([128, NT], FP, tag="swp")
                            nc.sync.dma_start(out=swp[0:64, :],
                                              in_=qraw[64:128, :])
                            nc.sync.dma_start(out=swp[64:128, :],
                                              in_=qraw[0:64, :])
                            t1 = rp.tile([128, NT], FP, tag="t1")
                            nc.vector.tensor_mul(t1, qraw, cosj)
                            t2 = rp.tile([128, NT], FP, tag="t2")
                            nc.vector.tensor_mul(t2, swp, sinj)
                            nc.vector.tensor_add(dst, t1, t2)

                sbcp_cm.__exit__(None, None, None)

                # ---------------- phase 2+3: attention + out-proj ----------
                with (
                    tc.tile_pool(name="wfc", bufs=1) as wfcp,
                    tc.tile_pool(name="att", bufs=18) as att,
                    tc.tile_pool(name="att2", bufs=4) as att2,
                    tc.tile_pool(name="abuf", bufs=2 * HL) as abufp,
                    tc.tile_pool(name="psS", bufs=3, space="PSUM") as psS,
                    tc.tile_pool(name="psA", bufs=2, space="PSUM") as psA,
                    tc.tile_pool(name="psD", bufs=1, space="PSUM") as psD,
                    tc.tile_pool(name="psF", bufs=2, space="PSUM") as psF,
                ):
                    cmask = []
                    for r in range(4):
                        cmt = wfcp.tile([128, NT], MD, tag=f"cmask{r}",
                                        name=f"cmask{r}")
                        nc.vector.memset(cmt, 1.0)
                        nc.gpsimd.affine_select(
                            out=cmt, in_=cmt,
                            compare_op=mybir.AluOpType.is_ge,
                            fill=0.0, base=-128 * r,
                            pattern=[[1, NT]], channel_multiplier=-1)
                        cmask.append(cmt)
                    wfc_sb = [wfcp.tile([128, E], MD, tag=f"wfc{h}",
                                        name=f"wfc{h}")
                              for h in range(HL)]
                    for h in range(HL):
                        nc.sync.dma_start(out=wfc_sb[h],
                                          in_=wfc_d[bass.ts(h, 128), :])
                    for j in range(NB if limit >= 2 else 0):
                        jc = bass.ts(j, NT)
                        ntk = 4 * (j + 1)          # allowed tk tiles
                        a_t = []
                        for h in range(HL):
                            # scores^T tiles + exp (+ causal mask on last 4)
                            et = []
                            for c in range(ntk):
                                psq = psS.tile([128, NT], FP, tag="psq")
                                nc.tensor.matmul(
                                    psq,
                                    k_sb[:, bass.ts(c, 128)],
                                    q_sb[h][:, jc],
                                    start=True, stop=True)
                                e = att.tile([128, NT], MD, tag="et")
                                nc.scalar.activation(e, psq, AF.Exp,
                                                     scale=SM_SCALE)
                                if c >= 4 * j:
                                    nc.vector.tensor_mul(e, e,
                                                         cmask[c - 4 * j])
                                et.append(e)
                            # denominator: ones-matmul column sums
                            psd = psD.tile([1, NT], FP, tag="psd")
                            for c in range(ntk):
                                nc.tensor.matmul(psd, ones_md, et[c],
                                                 start=(c == 0),
                                                 stop=(c == ntk - 1))
                            # A^T = sum_c V_c^T E_c
                            psa = psA.tile([128, NT], FP, tag="psa")
                            for c in range(ntk):
                                nc.tensor.matmul(psa, v_sb[c], et[c],
                                                 start=(c == 0),
                                                 stop=(c == ntk - 1))
                            # normalize by 1/denominator
                            rec = att2.tile([1, NT], FP, tag="rec")
                            nc.vector.reciprocal(rec, psd)
                            rbc = att2.tile([128, NT], FP, tag="rbc")
                            nc.gpsimd.partition_broadcast(rbc, rec)
                            ah = abufp.tile([128, NT], MD, tag="ah")
                            nc.vector.tensor_mul(ah, psa, rbc)
                            a_t.append(ah)
                        # out-proj partial: fcT[e, tq] += wfc^T a
                        for em in range(EM):
                            psf = psF.tile([128, NT], FP, tag="psf")
                            for h in range(HL):
                                nc.tensor.matmul(
                                    psf,
                                    wfc_sb[h][:, bass.ts(em, 128)],
                                    a_t[h],
                                    start=(h == 0), stop=(h == HL - 1))
                            fco = att2.tile([128, NT], CF, tag="fco")
                            nc.vector.tensor_copy(fco, psf)
                            nc.sync.dma_start(
                                out=fc_par[j][bass.ts(em, 128), :], in_=fco)
                        if collectives:
                            nc.gpsimd.collective_compute(
                                "AllReduce", mybir.AluOpType.add,
                                replica_groups=groups,
                                ins=[fc_par[j][:]], outs=[fc_ar[j][:]])
                        else:
                            nc.sync.dma_start(out=fc_ar[j][:],
                                              in_=fc_par[j][:])

            # ---------------- phase 4: FFN (+ folded h2 residual) ----------
            with (
                tc.tile_pool(name="h2c", bufs=KE) as h2p,
                tc.tile_pool(name="x2c", bufs=KE) as x2p,
                tc.tile_pool(name="f4", bufs=3) as f4,
                tc.tile_pool(name="wf", bufs=2) as wfp,
                tc.tile_pool(name="mt", bufs=FM) as mtp,
                tc.tile_pool(name="ps4s", bufs=1, space="PSUM") as ps4s,
                tc.tile_pool(name="ps4g", bufs=2, space="PSUM") as ps4g,
                tc.tile_pool(name="ps4h", bufs=2, space="PSUM") as ps4h,
                tc.tile_pool(name="ps4f", bufs=2, space="PSUM") as ps4f,
            ):
                for j in range(NB if limit >= 4 else 0):
                    jc = bass.ts(j, NT)
                    # h2 = x + attn_out; stats for second rmsnorm
                    h2 = []
                    ps_s2 = ps4s.tile([1, NT], FP, tag="ps_s2")
                    for k in range(KE):
                        ta = f4.tile([128, NT], CF, tag="ta")
                        nc.sync.dma_start(out=ta,
                                          in_=fc_ar[j][bass.ts(k, 128), :])
                        tx = f4.tile([128, NT], FP, tag="tx")
                        nc.sync.dma_start(out=tx, in_=xT_d[bass.ts(k, 128), jc])
                        h2k = h2p.tile([128, NT], FP, tag="h2")
                        nc.gpsimd.tensor_add(h2k, ta, tx)
                        h2.append(h2k)
                        sq = f4.tile([128, NT], MD, tag="sq4")
                        if k % 2 == 0:
                            nc.vector.tensor_mul(sq, h2k, h2k)
                        else:
                            nc.scalar.activation(sq, h2k, AF.Square)
                        nc.tensor.matmul(ps_s2, ones_md, sq,
                                         start=(k == 0), stop=(k == KE - 1))
                    srt2 = f4.tile([1, NT], FP, tag="srt2")
                    nc.scalar.activation(srt2, ps_s2, AF.Sqrt,
                                         bias=eps_t, scale=1.0 / E)
                    s2r = f4.tile([1, NT], FP, tag="s2r")
                    nc.vector.reciprocal(s2r, srt2)
                    s2bc = f4.tile([128, NT], FP, tag="s2bc")
                    nc.gpsimd.partition_broadcast(s2bc, s2r)
                    x2 = []
                    for k in range(KE):
                        x2k = x2p.tile([128, NT], MD, tag="x2")
                        nc.vector.tensor_mul(x2k, h2[k], s2bc)
                        x2.append(x2k)
                    # w1/w2 matmuls per ff m-tile
                    m_tiles = []
                    for m in range(FM):
                        w1m = wfp.tile([128, KE, 128], MD, tag="w1m")
                        nc.sync.dma_start(
                            out=w1m,
                            in_=w1_d[m].rearrange("(k p) c -> p k c", p=128))
                        w2m = wfp.tile([128, KE, 128], MD, tag="w2m")
                        nc.sync.dma_start(
                            out=w2m,
                            in_=w2_d[m].rearrange("(k p) c -> p k c", p=128))
                        psg1 = ps4g.tile([128, NT], FP, tag="psg1")
                        for k in range(KE):
                            nc.tensor.matmul(psg1, w1m[:, k, :], x2[k],
                                             start=(k == 0), stop=(k == KE - 1))
                        psg2 = ps4h.tile([128, NT], FP, tag="psg2")
                        for k in range(KE):
                            nc.tensor.matmul(psg2, w2m[:, k, :], x2[k],
                                             start=(k == 0), stop=(k == KE - 1))
                        g1s = f4.tile([128, NT], FP, tag="g1s")
                        nc.scalar.activation(g1s, psg1, AF.Sigmoid)
                        tmp = f4.tile([128, NT], FP, tag="tmp")
                        nc.vector.tensor_mul(tmp, g1s, psg1)
                        mt = mtp.tile([128, NT], MD, tag="mt")
                        nc.vector.tensor_mul(mt, tmp, psg2)
                        m_tiles.append(mt)
                    # w3 row-parallel partial, with h2/TP residual folded in
                    for em in range(EM):
                        w3t = wfp.tile([128, FM, 128], MD, tag="w3t")
                        nc.sync.dma_start(
                            out=w3t,
                            in_=w3_d[em].rearrange("(f p) c -> p f c", p=128))
                        psff = ps4f.tile([128, NT], FP, tag="psff")
                        for fm in range(FM):
                            nc.tensor.matmul(psff, w3t[:, fm, :], m_tiles[fm],
                                             start=(fm == 0),
                                             stop=(fm == FM - 1))
                        ffo = f4.tile([128, NT], FP, tag="ffo")
                        # ffo = h2/TP + psff   (residual folded into RS sum)
                        nc.vector.scalar_tensor_tensor(
                            out=ffo, in0=h2[em], scalar=1.0 / TP, in1=psff,
                            op0=mybir.AluOpType.mult, op1=mybir.AluOpType.add)
                        nc.sync.dma_start(out=ff_par[j][bass.ts(em, 128), :],
                                          in_=ffo)
                    if collectives:
                        nc.gpsimd.collective_compute(
                            "ReduceScatter", mybir.AluOpType.add,
                            replica_groups=groups,
                            ins=[ff_par[j][:]], outs=[rs_out[j][:]])
                    else:
                        nc.sync.dma_start(out=rs_out[j][:],
                                          in_=ff_par[j][0:ESH, :])

            # ---------------- final: out shard = rs_out ----------------
            for j in range(NB if limit >= 4 else 0):
                nc.sync.dma_start(out=out_d[:, bass.ts(j, NT)],
                                  in_=rs_out[j][:])

    nc.compile()
    return nc


# ------------------------------------------------------------- host wrapper
def _numpy_fallback(x, attention_mask, freqs_cis, w_qkv, w_fc, w1, w2, w3,
                    attn_norm_w, ff_norm_w):
    def rms(v, w):
        n = v * (1.0 / np.sqrt((v.astype(np.float32) ** 2).mean(-1,
                                                                keepdims=True)
                               + EPS))
        return n.astype(np.float32) * w

    def rope(v, f):
        b, s, h, d = v.shape
        vr = v.reshape(b, s, h, d // 2, 2)
        fr = f.reshape(1, s, 1, d // 2, 2)
        e = vr[..., 0] * fr[..., 0] - vr[..., 1] * fr[..., 1]
        o = vr[..., 1] * fr[..., 0] + vr[..., 0] * fr[..., 1]
        return np.stack([e, o], -1).reshape(b, s, h, d).astype(np.float32)

    bsz, s, _ = x.shape
    n_rep = NH // NKV
    h = rms(x, attn_norm_w)
    qkv = h @ w_qkv.T
    q, k, v = np.split(qkv, [NH * HD, NH * HD + NKV * HD], axis=-1)
    q = rope(q.reshape(bsz, s, NH, HD), freqs_cis).transpose(0, 2, 1, 3)
    k = rope(k.reshape(bsz, s, NKV, HD), freqs_cis).transpose(0, 2, 1, 3)
    v = v.reshape(bsz, s, NKV, HD).transpose(0, 2, 1, 3)
    k = np.repeat(k, n_rep, axis=1)
    v = np.repeat(v, n_rep, axis=1)
    sc = np.einsum("bhqd,bhkd->bhqk", q, k).astype(np.float32) * SM_SCALE
    sc = np.where(attention_mask[:, None] == 0, -np.inf, sc)
    sc = sc - sc.max(-1, keepdims=True)
    p = np.exp(sc)
    p = p / p.sum(-1, keepdims=True)
    at = np.einsum("bhqk,bhkd->bhqd", p, v).astype(np.float32)
    at = at.transpose(0, 2, 1, 3).reshape(bsz, s, -1) @ w_fc.T
    h = x + at
    g = rms(h, ff_norm_w)
    sil = g @ w1.T
    sil = sil / (1.0 + np.exp(-sil)) * (g @ w2.T)
    return (h + sil @ w3.T).astype(np.float32)


def _deinterleave(w):
    """Reorder head channel dim from (pair, 2) to [evens..., odds...]."""
    nh = w.shape[0] // HD
    wh = w.reshape(nh, HD, -1)
    return np.concatenate([wh[:, 0::2, :], wh[:, 1::2, :]], axis=1)


def kernel(**inputs):
    x = np.ascontiguousarray(np.asarray(inputs["x"], dtype=np.float32))
    mask = np.asarray(inputs["attention_mask"])
    freqs = np.ascontiguousarray(np.asarray(inputs["freqs_cis"],
                                            dtype=np.float32))
    w_qkv = np.asarray(inputs["w_qkv"], dtype=np.float32)
    w_fc = np.asarray(inputs["w_fc"], dtype=np.float32)
    w1 = np.asarray(inputs["w1"], dtype=np.float32)
    w2 = np.asarray(inputs["w2"], dtype=np.float32)
    w3 = np.asarray(inputs["w3"], dtype=np.float32)
    anw = np.asarray(inputs["attn_norm_w"], dtype=np.float32)
    fnw = np.asarray(inputs["ff_norm_w"], dtype=np.float32)

    tril = np.tril(np.ones((S, S), dtype=mask.dtype))
    if (x.shape != (B, S, E) or mask.shape != (B, S, S)
            or not all(np.array_equal(mask[b], tril) for b in range(B))):
        return _numpy_fallback(x, mask, freqs, w_qkv, w_fc, w1, w2, w3,
                               anw, fnw)

    key = (CFG["mmdt"], CFG["coll_fc"], CFG["collectives"])
    if key not in _prog_cache:
        _prog_cache[key] = _build_program()
    nc = _prog_cache[key]

    in_maps = _prep_in_maps(x, freqs, w_qkv, w_fc, w1, w2, w3, anw, fnw)

    res = run_bass_kernel_spmd(nc, in_maps, list(range(N_CORES)))
    return _assemble(res.results)


def _assemble(results):
    out = np.empty((B, S, E), dtype=np.float32)
    for b in range(DP):
        shard = np.concatenate(
            [results[b * TP + t]["outT"] for t in range(TP)], axis=0)
        out[b] = shard.T
    return out


def _prep_in_maps(x, freqs, w_qkv, w_fc, w1, w2, w3, anw, fnw):
    if CFG["mmdt"] == "bf16":
        import ml_dtypes
        wdt = ml_dtypes.bfloat16
    else:
        wdt = np.float32
    wq_all = _deinterleave(w_qkv[:NH * HD] * anw[None, :])        # (NH,HD,E)
    wk_all = _deinterleave(
        w_qkv[NH * HD:NH * HD + NKV * HD] * anw[None, :])         # (NKV,HD,E)
    wv_all = (w_qkv[NH * HD + NKV * HD:] * anw[None, :]).reshape(
        NKV, HD, E)                                               # (NKV,HD,E)
    w1f = w1 * fnw[None, :]
    w2f = w2 * fnw[None, :]
    # rope tables: stacked for deinterleaved layout; sin signed (-top, +bot)
    cosF = np.ascontiguousarray(
        np.concatenate([freqs[:, :, 0].T, freqs[:, :, 0].T], axis=0))
    sinS = np.ascontiguousarray(
        np.concatenate([-freqs[:, :, 1].T, freqs[:, :, 1].T], axis=0))

    in_maps = []
    for core in range(N_CORES):
        b, t = divmod(core, TP)
        qh = slice(t * HL, (t + 1) * HL)
        kv = (t * NKV) // TP
        wqkv_t = np.ascontiguousarray(np.concatenate(
            [wq_all[qh].reshape(QD, E).T,
             wk_all[kv].T, wv_all[kv].T], axis=1).astype(wdt))    # (E, QD+256)
        wfc_t = np.ascontiguousarray(
            w_fc[:, t * QD:(t + 1) * QD].T.astype(wdt))           # (QD, E)
        rows = slice(t * FFS, (t + 1) * FFS)
        w1_t = np.ascontiguousarray(
            w1f[rows].T.reshape(E, FM, 128).transpose(1, 0, 2).astype(wdt))
        w2_t = np.ascontiguousarray(
            w2f[rows].T.reshape(E, FM, 128).transpose(1, 0, 2).astype(wdt))
        w3_t = np.ascontiguousarray(
            w3[:, rows].T.reshape(FFS, EM, 128).transpose(1, 0, 2).astype(wdt))
        in_maps.append({
            "xT": np.ascontiguousarray(x[b].T),
            "xTb": np.ascontiguousarray(x[b].T.astype(wdt)),
            "wqkv": wqkv_t,
            "wfc": wfc_t,
            "w1": w1_t,
            "w2": w2_t,
            "w3": w3_t,
            "cosF": cosF,
            "sinS": sinS,
        })
    return in_maps



# revision 4
# speedup vs baseline: 14.9092x; 14.9092x over previous
"""Trainium2 Bass kernel for a dense transformer decoder layer.

Layer: RMSNorm -> QKV(+RoPE) -> causal GQA attention -> out-proj -> residual
       -> RMSNorm -> SwiGLU FFN -> residual
Shapes: B=2, S=2048, E=2048, NH=16, NKV=4, HD=128, FF=5632, fp32 I/O.

Sharding: DP over batch (2 replicas) x TP within replica (4 cores):
  - attention: TP by heads — each core owns NH/TP q-heads + 1 kv-head
    (column-parallel QKV).  The out-proj partials are computed TOKEN-major
    ([tokens, E]) so a per-token-block ReduceScatter over the 4-core group
    hands each core a fully-reduced 128-token slice.  Only collective in
    the kernel: 4 RS with 0.5 MB outputs (no AllReduce).
  - FFN: token-parallel — each core runs the FULL SwiGLU FFN (all of FF)
    for its S/TP = 512 tokens, weights streamed once.  The final residual
    is local, so each core directly produces its token-shard of the output
    and the host reassembles (no second collective).
  - on-chip activations are feature-major ("T layout": [feature, token]);
    the token-major RS slices are transposed back via PE transposes.

RMSNorm weight vectors are folded into the matmul weights on host; the
per-token 1/rms scale is computed via x^2 ones-matmuls and applied via the
rope tables (q/k) and a [128,1]-scalar multiply (v).  Softmax runs without
max-subtraction (logits here are O(1)); causal masking via block skipping
plus precomputed diagonal-block masks; ones-matmul for denominators.
RoPE's half-swap runs on the PE as a permutation matmul (no SBUF-SBUF
DMAs).

Matmul operand dtype is bf16 (fp32 for the rope permutation); accumulation
is fp32 in PSUM; both residual paths are fp32.
"""

import math
import os
import sys

import numpy as np

for _p in ("/opt/trn_rl_repo",):
    if _p not in sys.path and os.path.isdir(_p):
        sys.path.insert(0, _p)

import concourse.bass as bass
import concourse.tile as tile
from concourse import bacc, mybir
from concourse.bass_utils import run_bass_kernel_spmd

# ---------------------------------------------------------------- constants
B, S, E = 2, 2048, 2048
NH, NKV, HD, FF = 16, 4, 128, 5632
EPS = 1e-5
SM_SCALE = 1.0 / math.sqrt(float(E))  # reference scales by sqrt(embed_dim)

N_CORES = 8
TP = 4                      # tensor-parallel degree (cores per replica)
DP = N_CORES // TP          # data-parallel over batch
HL = NH // TP               # local q heads
QD = HL * HD                # local q dims
KE = E // 128               # embed k-tiles (16)
NT = 512                    # token block (matmul moving free dim)
NB = S // NT                # token blocks (4)
EM = E // 128               # embed out tiles (16)
FM = FF // 128              # FULL ffn m-tiles (44) — token-parallel FFN
TL = S // TP                # local tokens in FFN phase (512)

FP = mybir.dt.float32
AF = mybir.ActivationFunctionType

CFG = {
    "mmdt": os.environ.get("BASS_KERNEL_MMDT", "bf16"),
    "coll_fc": os.environ.get("BASS_KERNEL_COLL_FC", "bf16"),
    "collectives": os.environ.get("BASS_KERNEL_COLL", "1") == "1",
}

_DT = {"f32": mybir.dt.float32, "bf16": mybir.dt.bfloat16,
       "f32r": mybir.dt.float32r}

_prog_cache = {}


# ------------------------------------------------------------- device program
def _build_program(mmdt=None, coll_fc=None, collectives=None):
    mmdt = CFG["mmdt"] if mmdt is None else mmdt
    coll_fc = CFG["coll_fc"] if coll_fc is None else coll_fc
    collectives = CFG["collectives"] if collectives is None else collectives
    MD = _DT[mmdt]                       # matmul operand dtype
    WD = MD                              # weight dtype in DRAM
    CF = _DT[coll_fc]                    # RS payload dtype

    nc = bacc.Bacc("TRN2", target_bir_lowering=False, debug=False,
                   num_devices=N_CORES)

    # per-core fp32 x token-shard [E, TL] (col j*128+i <-> token
    # j*NT + t*128 + i of batch row b, for core = b*TP + t)
    xT_d = nc.dram_tensor("xT", [E, TL], FP, kind="ExternalInput").ap()
    xTb_d = nc.dram_tensor("xTb", [E, S], MD, kind="ExternalInput").ap()
    wqkv_d = nc.dram_tensor("wqkv", [E, QD + 2 * HD], WD,
                            kind="ExternalInput").ap()
    wfc_d = nc.dram_tensor("wfc", [QD, E], WD, kind="ExternalInput").ap()
    w12_d = nc.dram_tensor("w12", [FM, E, 256], WD, kind="ExternalInput").ap()
    w3_d = nc.dram_tensor("w3", [EM // 2, FF, 256], WD,
                          kind="ExternalInput").ap()
    cosF_d = nc.dram_tensor("cosF", [HD, S], FP, kind="ExternalInput").ap()
    sinS_d = nc.dram_tensor("sinS", [HD, S], FP, kind="ExternalInput").ap()
    ident_d = nc.dram_tensor("ident", [128, 128], MD, kind="ExternalInput").ap()
    perm_d = nc.dram_tensor("perm", [128, 128], FP, kind="ExternalInput").ap()

    out_d = nc.dram_tensor("outT", [E, TL], FP, kind="ExternalOutput").ap()

    groups = [[r * TP + t for t in range(TP)] for r in range(DP)]

    with tile.TileContext(nc) as tc:
        with (
            tc.tile_pool(name="const", bufs=1) as const,
            tc.tile_pool(name="dram", bufs=1, space="DRAM") as dram,
        ):
            # ---- internal DRAM buffers for the RS
            fc_par = [dram.tile([NT, E], CF, tag=f"fcp{j}", name=f"fcp{j}")
                      for j in range(NB)]
            rs_out = [dram.tile([128, E], CF, tag=f"rso{j}", name=f"rso{j}")
                      for j in range(NB)]

            # ---- constants
            ones_md = const.tile([128, 1], MD)
            nc.vector.memset(ones_md, 1.0)
            eps_t = const.tile([1, 1], FP)
            nc.vector.memset(eps_t, EPS)
            ident_sb = const.tile([128, 128], MD)
            nc.sync.dma_start(out=ident_sb, in_=ident_d)
            perm_sb = const.tile([128, 128], FP)
            nc.sync.dma_start(out=perm_sb, in_=perm_d)
            cos_all = const.tile([128, S], FP)
            nc.sync.dma_start(out=cos_all, in_=cosF_d)
            sin_all = const.tile([128, S], FP)
            nc.sync.dma_start(out=sin_all, in_=sinS_d)

            with (
                tc.tile_pool(name="qkstore", bufs=1) as qkstore,
                tc.tile_pool(name="vstore", bufs=1) as vstore,
            ):
                q_sb = [qkstore.tile([128, S], MD, tag=f"q{m}", name=f"q{m}")
                        for m in range(HL)]
                k_sb = qkstore.tile([128, S], MD, tag="kk")
                v_sb = [vstore.tile([128, HD], MD, tag=f"v{t}", name=f"v{t}")
                        for t in range(S // 128)]
                s_col = [vstore.tile([128, 1], FP, tag=f"sc{t}", name=f"sc{t}")
                         for t in range(S // 128)]
                s_dram = dram.tile([1, S], FP, tag="s_dram", name="s_dram")

                # ------------- phase 0+1: stats + qkv + rope, per block -----
                with (
                    tc.tile_pool(name="wq", bufs=1) as wqp,
                    tc.tile_pool(name="xh", bufs=2) as xhp,
                    tc.tile_pool(name="rp", bufs=3) as rp,
                    tc.tile_pool(name="sb0", bufs=2) as sb0,
                    tc.tile_pool(name="ps1", bufs=2, space="PSUM") as ps1,
                    tc.tile_pool(name="ps1v", bufs=2, space="PSUM") as ps1v,
                    tc.tile_pool(name="psw", bufs=2, space="PSUM") as psw,
                    tc.tile_pool(name="ps0", bufs=1, space="PSUM") as ps0,
                ):
                    wq_sb = wqp.tile([128, KE, QD + 2 * HD], MD, tag="wq")
                    nc.sync.dma_start(
                        out=wq_sb,
                        in_=wqkv_d.rearrange("(k p) c -> p k c", p=128))
                    for j in range(NB):
                        jc = bass.ts(j, NT)
                        xh = xhp.tile([128, KE, NT], MD, tag="xh")
                        nc.sync.dma_start(
                            out=xh,
                            in_=xTb_d[:, jc].rearrange("(k p) n -> p k n",
                                                       p=128))
                        # token 1/rms for this block
                        ps_s = ps0.tile([1, NT], FP, tag="ps_s")
                        for k in range(KE):
                            sq = sb0.tile([128, NT], MD, tag="sq")
                            if k % 2 == 0:
                                nc.vector.tensor_mul(sq, xh[:, k], xh[:, k])
                            else:
                                nc.scalar.activation(sq, xh[:, k], AF.Square)
                            nc.tensor.matmul(ps_s, ones_md, sq,
                                             start=(k == 0), stop=(k == KE - 1))
                        srt = sb0.tile([1, NT], FP, tag="srt")
                        nc.scalar.activation(srt, ps_s, AF.Sqrt,
                                             bias=eps_t, scale=1.0 / E)
                        s_row = sb0.tile([1, NT], FP, tag="s_row")
                        nc.vector.reciprocal(s_row, srt)
                        sbc = sb0.tile([128, NT], FP, tag="sbc")
                        nc.gpsimd.partition_broadcast(sbc, s_row)
                        # column layout of the scale for the v tiles
                        nc.sync.dma_start(out=s_dram[:, jc], in_=s_row)
                        for tt in range(4):
                            nc.sync.dma_start(
                                out=s_col[4 * j + tt],
                                in_=s_dram[0, bass.ts(4 * j + tt, 128)]
                                .rearrange("(p one) -> p one", one=1))
                        # rope tables scaled by 1/rms
                        cosj = rp.tile([128, NT], FP, tag="cosj")
                        nc.vector.tensor_mul(cosj, cos_all[:, jc], sbc)
                        sinj = rp.tile([128, NT], FP, tag="sinj")
                        nc.vector.tensor_mul(sinj, sin_all[:, jc], sbc)
                        # v tiles (token-major)
                        for tt in range(4):
                            psv = ps1v.tile([128, HD], FP, tag="psv")
                            for k in range(KE):
                                nc.tensor.matmul(
                                    psv,
                                    xh[:, k, bass.ts(tt, 128)],
                                    wq_sb[:, k, QD + HD:QD + 2 * HD],
                                    start=(k == 0), stop=(k == KE - 1))
                            nc.vector.tensor_scalar_mul(
                                v_sb[4 * j + tt], psv, s_col[4 * j + tt])
                        # q (HL tiles) + k (1 tile), feature-major + rope
                        for m in range(HL + 1):
                            pq = ps1.tile([128, NT], FP, tag="pq")
                            for k in range(KE):
                                nc.tensor.matmul(
                                    pq,
                                    wq_sb[:, k, bass.ts(m, 128)],
                                    xh[:, k],
                                    start=(k == 0), stop=(k == KE - 1))
                            dst = (q_sb[m] if m < HL else k_sb)[:, jc]
                            qraw = rp.tile([128, NT], FP, tag="qraw")
                            nc.scalar.activation(qraw, pq, AF.Copy)
                            ps_sw = psw.tile([128, NT], FP, tag="ps_sw")
                            nc.tensor.matmul(ps_sw, perm_sb, qraw,
                                             start=True, stop=True)
                            t1 = rp.tile([128, NT], FP, tag="t1")
                            nc.vector.tensor_mul(t1, qraw, cosj)
                            t2 = rp.tile([128, NT], FP, tag="t2")
                            nc.vector.tensor_mul(t2, ps_sw, sinj)
                            nc.vector.tensor_add(dst, t1, t2)

                # ------------- phase 2+3: attention + token-major out-proj --
                with (
                    tc.tile_pool(name="wfc", bufs=1) as wfcp,
                    tc.tile_pool(name="att", bufs=18) as att,
                    tc.tile_pool(name="att2", bufs=4) as att2,
                    tc.tile_pool(name="abuf", bufs=2 * HL) as abufp,
                    tc.tile_pool(name="fcsb", bufs=2) as fcsbp,
                    tc.tile_pool(name="psS", bufs=3, space="PSUM") as psS,
                    tc.tile_pool(name="psA", bufs=2, space="PSUM") as psA,
                    tc.tile_pool(name="psD", bufs=1, space="PSUM") as psD,
                    tc.tile_pool(name="psF", bufs=2, space="PSUM") as psF,
                ):
                    cmask = []
                    for r in range(4):
                        cmt = wfcp.tile([128, NT], MD, tag=f"cmask{r}",
                                        name=f"cmask{r}")
                        nc.vector.memset(cmt, 1.0)
                        nc.gpsimd.affine_select(
                            out=cmt, in_=cmt,
                            compare_op=mybir.AluOpType.is_ge,
                            fill=0.0, base=-128 * r,
                            pattern=[[1, NT]], channel_multiplier=-1)
                        cmask.append(cmt)
                    wfc_sb = wfcp.tile([128, HL, E], MD, tag="wfc")
                    nc.sync.dma_start(
                        out=wfc_sb,
                        in_=wfc_d.rearrange("(h p) e -> p h e", p=128))
                    for j in range(NB):
                        jc = bass.ts(j, NT)
                        ntk = 4 * (j + 1)          # allowed tk tiles
                        a_t = []
                        for h in range(HL):
                            et = []
                            for c in range(ntk):
                                psq = psS.tile([128, NT], FP, tag="psq")
                                nc.tensor.matmul(
                                    psq,
                                    k_sb[:, bass.ts(c, 128)],
                                    q_sb[h][:, jc],
                                    start=True, stop=True)
                                e = att.tile([128, NT], MD, tag="et")
                                nc.scalar.activation(e, psq, AF.Exp,
                                                     scale=SM_SCALE)
                                if c >= 4 * j:
                                    nc.vector.tensor_mul(e, e,
                                                         cmask[c - 4 * j])
                                et.append(e)
                            psd = psD.tile([1, NT], FP, tag="psd")
                            for c in range(ntk):
                                nc.tensor.matmul(psd, ones_md, et[c],
                                                 start=(c == 0),
                                                 stop=(c == ntk - 1))
                            psa = psA.tile([128, NT], FP, tag="psa")
                            for c in range(ntk):
                                nc.tensor.matmul(psa, v_sb[c], et[c],
                                                 start=(c == 0),
                                                 stop=(c == ntk - 1))
                            rec = att2.tile([1, NT], FP, tag="rec")
                            nc.vector.reciprocal(rec, psd)
                            rbc = att2.tile([128, NT], FP, tag="rbc")
                            nc.gpsimd.partition_broadcast(rbc, rec)
                            ah = abufp.tile([128, NT], MD, tag="ah")
                            nc.vector.tensor_mul(ah, psa, rbc)
                            a_t.append(ah)
                        # token-major out-proj partial: fc[tq, e]
                        for tt in range(4):
                            fc_sb = fcsbp.tile([128, E], CF, tag="fcsb")
                            for ec in range(E // NT):
                                psf = psF.tile([128, NT], FP, tag="psf")
                                for h in range(HL):
                                    nc.tensor.matmul(
                                        psf,
                                        a_t[h][:, bass.ts(tt, 128)],
                                        wfc_sb[:, h, bass.ts(ec, NT)],
                                        start=(h == 0), stop=(h == HL - 1))
                                nc.vector.tensor_copy(
                                    fc_sb[:, bass.ts(ec, NT)], psf)
                            nc.sync.dma_start(
                                out=fc_par[j][bass.ts(tt, 128), :], in_=fc_sb)
                        if collectives:
                            nc.gpsimd.collective_compute(
                                "ReduceScatter", mybir.AluOpType.add,
                                replica_groups=groups,
                                ins=[fc_par[j][:]], outs=[rs_out[j][:]])
                        else:
                            nc.sync.dma_start(out=rs_out[j][:],
                                              in_=fc_par[j][0:128, :])

            # ------------- phase 4: token-parallel FFN ---------------------
            # Full FF for this core's TL=512 tokens (4 chunks of 128, one per
            # block, via the RS).  h2 = x + attnT, rmsnorm, SwiGLU, + h2.
            with (
                tc.tile_pool(name="h2c", bufs=1) as h2p,
                tc.tile_pool(name="rssb", bufs=2) as rssbp,
                tc.tile_pool(name="f4", bufs=2) as f4,
                tc.tile_pool(name="wf", bufs=2) as wfp,
                tc.tile_pool(name="w3p", bufs=2) as w3p,
                tc.tile_pool(name="ps4t", bufs=1, space="PSUM") as ps4t,
                tc.tile_pool(name="ps4s", bufs=1, space="PSUM") as ps4s,
                tc.tile_pool(name="ps4g", bufs=2, space="PSUM") as ps4g,
                tc.tile_pool(name="ps4h", bufs=2, space="PSUM") as ps4h,
                tc.tile_pool(name="ps4f", bufs=2, space="PSUM") as ps4f,
            ):
                # h2 starts as the local fp32 x shard; attn added in place
                h2t = h2p.tile([128, KE, TL], FP, tag="h2t")
                nc.sync.dma_start(
                    out=h2t, in_=xT_d.rearrange("(k p) n -> p k n", p=128))
                x2t = h2p.tile([128, KE, TL], MD, tag="x2t")
                for j in range(NB):
                    rs_sb = rssbp.tile([128, E], CF, tag="rs")
                    nc.sync.dma_start(out=rs_sb, in_=rs_out[j][:])
                    for c in range(KE):
                        pst = ps4t.tile([128, 128], MD, tag="pst")
                        nc.tensor.transpose(
                            pst, rs_sb[:, bass.ts(c, 128)], ident_sb)
                        dst = h2t[:, c, bass.ts(j, 128)]
                        nc.vector.tensor_add(dst, dst, pst)
                ps_s2 = ps4s.tile([1, TL], FP, tag="ps_s2")
                for c in range(KE):
                    sq = f4.tile([128, TL], MD, tag="sq4")
                    if c % 2 == 0:
                        nc.vector.tensor_mul(sq, h2t[:, c], h2t[:, c])
                    else:
                        nc.scalar.activation(sq, h2t[:, c], AF.Square)
                    nc.tensor.matmul(ps_s2, ones_md, sq,
                                     start=(c == 0), stop=(c == KE - 1))
                srt2 = f4.tile([1, TL], FP, tag="srt2")
                nc.scalar.activation(srt2, ps_s2, AF.Sqrt,
                                     bias=eps_t, scale=1.0 / E)
                s2r = f4.tile([1, TL], FP, tag="s2r")
                nc.vector.reciprocal(s2r, srt2)
                s2bc = f4.tile([128, TL], FP, tag="s2bc")
                nc.gpsimd.partition_broadcast(s2bc, s2r)
                for c in range(KE):
                    nc.vector.tensor_mul(x2t[:, c], h2t[:, c], s2bc)
                # SwiGLU hidden: full FF, FM=44 m-tiles
                mt_t = h2p.tile([128, FM, TL], MD, tag="mt_t")
                for m in range(FM):
                    w12m = wfp.tile([128, KE, 256], WD, tag="w12m")
                    nc.sync.dma_start(
                        out=w12m,
                        in_=w12_d[m].rearrange("(k p) c -> p k c", p=128))
                    psg1 = ps4g.tile([128, TL], FP, tag="psg1")
                    for k in range(KE):
                        nc.tensor.matmul(psg1, w12m[:, k, 0:128], x2t[:, k],
                                         start=(k == 0), stop=(k == KE - 1))
                    psg2 = ps4h.tile([128, TL], FP, tag="psg2")
                    for k in range(KE):
                        nc.tensor.matmul(psg2, w12m[:, k, 128:256], x2t[:, k],
                                         start=(k == 0), stop=(k == KE - 1))
                    g1s = f4.tile([128, TL], FP, tag="g1s")
                    nc.scalar.activation(g1s, psg1, AF.Sigmoid)
                    tmp = f4.tile([128, TL], FP, tag="tmp")
                    nc.vector.tensor_mul(tmp, g1s, psg1)
                    nc.vector.tensor_mul(mt_t[:, m], tmp, psg2)
                # w3 + residual -> output shard
                for em in range(EM):
                    w3t = w3p.tile([128, FM, 128], WD, tag="w3t")
                    nc.sync.dma_start(
                        out=w3t,
                        in_=w3_d[em].rearrange("(f p) c -> p f c", p=128))
                    psff = ps4f.tile([128, TL], FP, tag="psff")
                    for fm in range(FM):
                        nc.tensor.matmul(psff, w3t[:, fm], mt_t[:, fm],
                                         start=(fm == 0),
                                         stop=(fm == FM - 1))
                    ffo = f4.tile([128, TL], FP, tag="ffo")
                    nc.vector.tensor_add(ffo, h2t[:, em], psff)
                    nc.sync.dma_start(out=out_d[bass.ts(em, 128), :], in_=ffo)

    nc.compile()
    return nc


# ------------------------------------------------------------- host wrapper
def _numpy_fallback(x, attention_mask, freqs_cis, w_qkv, w_fc, w1, w2, w3,
                    attn_norm_w, ff_norm_w):
    def rms(v, w):
        n = v * (1.0 / np.sqrt((v.astype(np.float32) ** 2).mean(-1,
                                                                keepdims=True)
                               + EPS))
        return n.astype(np.float32) * w

    def rope(v, f):
        b, s, h, d = v.shape
        vr = v.reshape(b, s, h, d // 2, 2)
        fr = f.reshape(1, s, 1, d // 2, 2)
        e = vr[..., 0] * fr[..., 0] - vr[..., 1] * fr[..., 1]
        o = vr[..., 1] * fr[..., 0] + vr[..., 0] * fr[..., 1]
        return np.stack([e, o], -1).reshape(b, s, h, d).astype(np.float32)

    bsz, s, _ = x.shape
    n_rep = NH // NKV
    h = rms(x, attn_norm_w)
    qkv = h @ w_qkv.T
    q, k, v = np.split(qkv, [NH * HD, NH * HD + NKV * HD], axis=-1)
    q = rope(q.reshape(bsz, s, NH, HD), freqs_cis).transpose(0, 2, 1, 3)
    k = rope(k.reshape(bsz, s, NKV, HD), freqs_cis).transpose(0, 2, 1, 3)
    v = v.reshape(bsz, s, NKV, HD).transpose(0, 2, 1, 3)
    k = np.repeat(k, n_rep, axis=1)
    v = np.repeat(v, n_rep, axis=1)
    sc = np.einsum("bhqd,bhkd->bhqk", q, k).astype(np.float32) * SM_SCALE
    sc = np.where(attention_mask[:, None] == 0, -np.inf, sc)
    sc = sc - sc.max(-1, keepdims=True)
    p = np.exp(sc)
    p = p / p.sum(-1, keepdims=True)
    at = np.einsum("bhqk,bhkd->bhqd", p, v).astype(np.float32)
    at = at.transpose(0, 2, 1, 3).reshape(bsz, s, -1) @ w_fc.T
    h = x + at
    g = rms(h, ff_norm_w)
    sil = g @ w1.T
    sil = sil / (1.0 + np.exp(-sil)) * (g @ w2.T)
    return (h + sil @ w3.T).astype(np.float32)


def _deinterleave(w):
    """Reorder head channel dim from (pair, 2) to [evens..., odds...]."""
    nh = w.shape[0] // HD
    wh = w.reshape(nh, HD, -1)
    return np.concatenate([wh[:, 0::2, :], wh[:, 1::2, :]], axis=1)


def kernel(**inputs):
    x = np.ascontiguousarray(np.asarray(inputs["x"], dtype=np.float32))
    mask = np.asarray(inputs["attention_mask"])
    freqs = np.ascontiguousarray(np.asarray(inputs["freqs_cis"],
                                            dtype=np.float32))
    w_qkv = np.asarray(inputs["w_qkv"], dtype=np.float32)
    w_fc = np.asarray(inputs["w_fc"], dtype=np.float32)
    w1 = np.asarray(inputs["w1"], dtype=np.float32)
    w2 = np.asarray(inputs["w2"], dtype=np.float32)
    w3 = np.asarray(inputs["w3"], dtype=np.float32)
    anw = np.asarray(inputs["attn_norm_w"], dtype=np.float32)
    fnw = np.asarray(inputs["ff_norm_w"], dtype=np.float32)

    tril = np.tril(np.ones((S, S), dtype=mask.dtype))
    if (x.shape != (B, S, E) or mask.shape != (B, S, S)
            or not all(np.array_equal(mask[b], tril) for b in range(B))):
        return _numpy_fallback(x, mask, freqs, w_qkv, w_fc, w1, w2, w3,
                               anw, fnw)

    key = (CFG["mmdt"], CFG["coll_fc"], CFG["collectives"])
    if key not in _prog_cache:
        _prog_cache[key] = _build_program()
    nc = _prog_cache[key]

    in_maps = _prep_in_maps(x, freqs, w_qkv, w_fc, w1, w2, w3, anw, fnw)

    res = run_bass_kernel_spmd(nc, in_maps, list(range(N_CORES)))
    return _assemble(res.results)


def _assemble(results):
    out = np.empty((B, S, E), dtype=np.float32)
    for b in range(DP):
        for t in range(TP):
            sh = results[b * TP + t]["outT"]          # [E, TL]
            for j in range(NB):
                lo = j * NT + t * 128
                out[b, lo:lo + 128, :] = sh[:, j * 128:(j + 1) * 128].T
    return out


def _prep_in_maps(x, freqs, w_qkv, w_fc, w1, w2, w3, anw, fnw):
    if CFG["mmdt"] == "bf16":
        import ml_dtypes
        wdt = ml_dtypes.bfloat16
    else:
        wdt = np.float32
    wq_all = _deinterleave(w_qkv[:NH * HD] * anw[None, :])        # (NH,HD,E)
    wk_all = _deinterleave(
        w_qkv[NH * HD:NH * HD + NKV * HD] * anw[None, :])         # (NKV,HD,E)
    wv_all = (w_qkv[NH * HD + NKV * HD:] * anw[None, :]).reshape(
        NKV, HD, E)                                               # (NKV,HD,E)
    w1f = (w1 * fnw[None, :]).astype(np.float32)
    w2f = (w2 * fnw[None, :]).astype(np.float32)
    # w12: [FM, E, 256] — per m-tile, w1 block then w2 block
    w1T = np.ascontiguousarray(w1f.T.reshape(E, FM, 128))
    w2T = np.ascontiguousarray(w2f.T.reshape(E, FM, 128))
    w12 = np.ascontiguousarray(
        np.concatenate([w1T, w2T], axis=2).transpose(1, 0, 2).astype(wdt))
    w3T = np.ascontiguousarray(
        w3.T.reshape(FF, EM, 128).transpose(1, 0, 2).astype(wdt))  # (EM,FF,128)
    # rope tables: stacked for deinterleaved layout; sin signed (-top, +bot)
    cosF = np.ascontiguousarray(
        np.concatenate([freqs[:, :, 0].T, freqs[:, :, 0].T], axis=0))
    sinS = np.ascontiguousarray(
        np.concatenate([-freqs[:, :, 1].T, freqs[:, :, 1].T], axis=0))
    ident = np.eye(128, dtype=wdt)
    perm = np.roll(np.eye(128, dtype=np.float32), 64, axis=0)

    xT = np.ascontiguousarray(x.transpose(0, 2, 1))               # (B, E, S)

    in_maps = []
    for core in range(N_CORES):
        b, t = divmod(core, TP)
        qh = slice(t * HL, (t + 1) * HL)
        kv = (t * NKV) // TP
        wqkv_t = np.ascontiguousarray(np.concatenate(
            [wq_all[qh].reshape(QD, E).T,
             wk_all[kv].T, wv_all[kv].T], axis=1).astype(wdt))    # (E, QD+256)
        wfc_t = np.ascontiguousarray(
            w_fc[:, t * QD:(t + 1) * QD].T.astype(wdt))           # (QD, E)
        # this core's fp32 x token-shard: [E, TL], col j*128+i
        xsh = np.ascontiguousarray(
            xT[b].reshape(E, NB, TP, 128)[:, :, t, :].reshape(E, TL))
        in_maps.append({
            "xT": xsh,
            "xTb": np.ascontiguousarray(xT[b].astype(wdt)),
            "wqkv": wqkv_t,
            "wfc": wfc_t,
            "w12": w12,
            "w3": w3T,
            "cosF": cosF,
            "sinS": sinS,
            "ident": ident,
            "perm": perm,
        })
    return in_maps
